# revision 14
# baseline (speedup 1.0000x reference)
"""Depthwise causal Conv1d (k=4) + SiLU on 8 Trainium2 NeuronCores.

Problem: x [4, 4096, 2048] f32, w [2048, 4] f32,
out[b, t, d] = silu(sum_j w[d, j] * x[b, t - 3 + j, d])   (zero-padded left).

Sharding: 8 cores = 4 batches x 2 channel-halves. Depthwise conv is
independent per channel, so channel sharding needs no halo exchange.

Layout: each core receives its shard host-transposed to [channels, time]
(channels on SBUF partitions). The per-channel weight w[d, j] is a
per-partition scalar and the causal time shifts are free-dim AP offsets
into one loaded tile.

The kernel is HBM-bandwidth-bound: ~16.4 MB of fp16 I/O per core against
the ~27 GiB/s-per-SDMA-engine line rate (16 engines, ~44 us of streaming),
with compute only just fitting under the DMA window. Schedule:
 - All loads queue back to back at the head of the sync HWDGE ring
   (pure reads, no stalls); per-block stores queue on the SAME ring
   afterwards with plain data deps on their output tiles, so store
   descriptor-gen interleaves with the tail of the load stream and
   store HBM traffic overlaps compute instead of serializing after it.
 - diag(w_j) stationaries for the PE path are host-built and loaded
   as ONE packed 0.64 MB slab right after w (a single dispatch slot;
   building them on-chip costs critical DVE time, and GpSimd - though
   idle - must stay silent: any GpSimd streaming collides with DVE on
   the shared SBUF port and roughly halves DVE throughput).
 - Compute is spread so no engine exceeds the DMA window: blocks
   1,3,5,7 + half of 6 run on the TensorEngine as diag(w_j) matmuls
   accumulating the 4 taps in PSUM (j-inner per 512-col chunk,
   [128,1024] PSUM tiles 4 deep so the PE stays ahead of ACT's drain);
   blocks 0,2,4 + half of 6 run elementwise on DVE (shift-rebased
   products, pair-add tree), with the q1 product of the earliest
   halves on ACT (idle before SiLUs pile up). ACT does SiLU for
   everything: 1024-col chunks from PSUM, 2048-col from SBUF.

Precision: x and the output are host-cast fp16 (halves HBM traffic both
ways); products and adds stay fp16 (PE accumulates fp32 in PSUM); SiLU
computes fp32-internally on ACT. End-to-end relative error ~5e-4.
"""

import sys
import types

import numpy as np

import concourse.bass as bass
import concourse.bacc as bacc
import concourse.mybir as mybir
from concourse.tile import TileContext
from concourse.bass_utils import run_bass_kernel_spmd


def _ensure_ntff_hook():
    """bass_utils imports antenv.axon_hooks when BASS_TRACE is set; that
    module is absent on this image. Install a shim so tracing works when
    possible and degrades gracefully (instead of crashing) when not."""
    try:
        import antenv.axon_hooks  # noqa: F401

        return
    except ImportError:
        pass
    try:
        import antenv

        hook = None
        try:
            if "/root/.axon_site" not in sys.path:
                sys.path.insert(0, "/root/.axon_site")
            from trn_agent_boot.trn_boot import _ntff_profile_via_ctypes

            hook = _ntff_profile_via_ctypes("/opt/axon/libaxon_pjrt.so")
        except Exception:
            hook = None
        mod = types.ModuleType("antenv.axon_hooks")
        mod._hook = hook
        mod.get_axon_ntff_profile_hook = lambda: mod._hook
        mod.set_axon_ntff_profile_hook = lambda h: setattr(mod, "_hook", h)
        sys.modules["antenv.axon_hooks"] = mod
        antenv.axon_hooks = mod
    except Exception:
        pass


_ensure_ntff_hook()

B, L, D = 4, 4096, 2048
K = 4
PAD = K - 1
N_CORES = 8
DH = D // 2            # channels per core
NBLK = DH // 128       # 128-partition channel blocks per core
ROWW = 4128            # DRAM row stride (fp16 elems): 64B-aligned rows
HALF = L // 2
PQ = 1024              # PSUM tile width / PE silu chunk

MID_DT = mybir.dt.float16
PE_FULL = (1, 3, 5, 7)   # full blocks on the TensorEngine
PE_HALF_BLK = 6          # block 6: h0 on DVE, h1 on the PE
DIAG_BLKS = (1, 3, 5, 7, 6)

_cache = {}


def _build_bass():
    nc = bacc.Bacc()
    xt = nc.dram_tensor("xt", [DH, ROWW], MID_DT, kind="ExternalInput")
    wt = nc.dram_tensor("wt", [128, NBLK * K], mybir.dt.float32, kind="ExternalInput")
    # host-built diag(w_j) stationaries for the PE path, packed as one
    # [128, 5*K*128] fp16 slab (one DMA dispatch slot; building these
    # on-chip costs critical DVE time, and GpSimd - though idle - must
    # not run: any GpSimd activity collides with DVE on the shared SBUF
    # port and roughly halves DVE throughput while it streams).
    wdt = nc.dram_tensor(
        "wdt", [128, len(DIAG_BLKS) * K * 128], MID_DT, kind="ExternalInput"
    )
    ot = nc.dram_tensor("ot", [DH, L], MID_DT, kind="ExternalOutput")
    f32 = mybir.dt.float32
    silu = mybir.ActivationFunctionType.Silu

    with TileContext(nc) as tc:
        with tc.tile_pool(name="pool", bufs=2) as pool, \
             tc.tile_pool(name="psum", bufs=4, space="PSUM") as psum_pool:
            # w + diag slabs lead the sync ring so the PE can start as
            # soon as block 1's x lands.
            w = pool.tile([128, NBLK * K], f32, tag="w", bufs=1)
            nc.sync.dma_start(out=w[:], in_=wt[:, :])
            wd = pool.tile(
                [128, len(DIAG_BLKS) * K * 128], MID_DT, tag="wd", bufs=1
            )
            nc.sync.dma_start(out=wd[:], in_=wdt[:, :])
            # Warmup: a tiny Silu forces the silu activation-table set to
            # load during the initial DMA wait; it is the only table load
            # in the whole kernel.
            warm = pool.tile([128, 2], MID_DT, tag="warm", bufs=1)
            nc.vector.memset(warm[:], 0.0)
            nc.scalar.activation(warm[:], warm[:], silu)

            # All x loads up front, back to back on the sync ring. The
            # first two blocks load in pieces so compute starts sooner.
            # Block 1 (a PE block) loads FIRST: the PE chain is the
            # critical tail, so it must start as early as possible; DVE
            # (block 0) follows and has more slack. x7 loads before x6
            # so the PE is fed in processing order.
            LOAD_ORDER = [1, 0, 2, 3, 4, 5, 7, 6]
            N_PIECES = {0: 2, 1: 2}
            xts = {}
            for blk in LOAD_ORDER:
                x = pool.tile([128, L + PAD + 1], MID_DT, tag="x", bufs=NBLK)
                n_p = N_PIECES.get(blk, 1)
                step = L // n_p
                cuts = [0] + [p * step + PAD for p in range(1, n_p)] + [L + PAD]
                for t0, t1 in zip(cuts[:-1], cuts[1:]):
                    nc.sync.dma_start(
                        out=x[:, t0:t1],
                        in_=xt[blk * 128 : (blk + 1) * 128, t0:t1],
                    )
                xts[blk] = x

            def lw_of(blk, j):
                c = (DIAG_BLKS.index(blk) * K + j) * 128
                return wd[:, c : c + 128]

            def pe_part(blk, x, o, t0, size):
                # TensorEngine path for [t0, t0+size): accumulate the 4
                # diag(w_j) matmuls per 512-col PSUM chunk (shift =
                # free-dim offset on the moving operand), SiLU from PSUM.
                # 1024-col PSUM quarters, 4 deep, so the PE stays 3 ahead
                # of ACT's PSUM drain.
                for q0 in range(t0, t0 + size, PQ):
                    ps = psum_pool.tile([128, PQ], f32, tag="ps", bufs=4)
                    for cc in range(PQ // 512):
                        for j in range(K):
                            nc.tensor.matmul(
                                ps[:, cc * 512 : (cc + 1) * 512],
                                lw_of(blk, j),
                                x[:, q0 + cc * 512 + j : q0 + cc * 512 + j + 512],
                                start=(j == 0),
                                stop=(j == K - 1),
                            )
                    nc.scalar.activation(
                        o[:, q0 : q0 + PQ], ps[:], silu
                    )

            def dve_half(blk, x, o, h0, n_act=0):
                # Elementwise path for [h0, h0+HALF): qe holds the
                # even-shift products [q0 | q2], qo the odd [q1 | q3]
                # (n_act of the odd ones on ACT to relieve DVE), pair-add
                # + final add on DVE, one 2048-col SiLU on ACT.
                # Shift-rebased: q_j[:, t] = w_j * x[:, h0 + t + j].
                wj = lambda j: w[:, blk * K + j : blk * K + j + 1]
                qe = pool.tile([128, 2, HALF], MID_DT, tag="qe", bufs=3)
                qo = pool.tile([128, 2, HALF], MID_DT, tag="qo", bufs=3)
                if n_act >= 1:
                    nc.scalar.mul(qo[:, 0, :], x[:, h0 + 1 : h0 + 1 + HALF], wj(1))
                else:
                    nc.vector.tensor_scalar_mul(
                        qo[:, 0, :], x[:, h0 + 1 : h0 + 1 + HALF], wj(1)
                    )
                nc.vector.tensor_scalar_mul(qe[:, 0, :], x[:, h0 : h0 + HALF], wj(0))
                if n_act >= 2:
                    nc.scalar.mul(qo[:, 1, :], x[:, h0 + 3 : h0 + 3 + HALF], wj(3))
                else:
                    nc.vector.tensor_scalar_mul(
                        qo[:, 1, :], x[:, h0 + 3 : h0 + 3 + HALF], wj(3)
                    )
                nc.vector.tensor_scalar_mul(qe[:, 1, :], x[:, h0 + 2 : h0 + 2 + HALF], wj(2))
                nc.vector.tensor_add(qe[:, :, :], qe[:, :, :], qo[:, :, :])
                nc.vector.tensor_add(qe[:, 0, :], qe[:, 0, :], qe[:, 1, :])
                nc.scalar.activation(o[:, h0 : h0 + HALF], qe[:, 0, :], silu)

            # Per-block compute. Emission order = per-engine queue order:
            # block 7 before 6 so the ACT SiLU queue tail matches
            # readiness (x6 is the last load). ACT takes the q1 product
            # of the earliest DVE halves (it is idle before SiLUs pile
            # up; DVE is the tighter budget).
            ACT_ODD = {(0, 0): 1, (0, 1): 1, (2, 0): 1}
            os_ = {}
            for blk in [0, 1, 2, 3, 4, 5, 7, 6]:
                x = xts[blk]
                o = pool.tile([128, L], MID_DT, tag="o", bufs=NBLK)
                os_[blk] = o
                if blk in PE_FULL:
                    pe_part(blk, x, o, 0, HALF)
                    pe_part(blk, x, o, HALF, HALF)
                elif blk == PE_HALF_BLK:
                    dve_half(blk, x, o, 0)
                    pe_part(blk, x, o, HALF, HALF)
                else:
                    for h in (0, 1):
                        dve_half(blk, x, o, h * HALF, ACT_ODD.get((blk, h), 0))

            # Stores: one full block per dma_start, on the sync ring
            # behind all loads, each depending only on its output tile
            # (last writer: that block's SiLUs).
            for blk in [0, 1, 2, 3, 4, 5, 7, 6]:
                nc.sync.dma_start(
                    out=ot[blk * 128 : (blk + 1) * 128, :], in_=os_[blk][:, :]
                )
    nc.compile()
    return nc


def _shard_inputs(x, w):
    in_maps = []
    for core in range(N_CORES):
        b, half = divmod(core, 2)
        d0 = half * DH
        xt = np.zeros((DH, ROWW), dtype=np.float16)
        xt[:, PAD : PAD + L] = x[b, :, d0 : d0 + DH].T.astype(np.float16)
        # w rows for this shard, rearranged so partition p holds the K
        # weights of channel blk*128 + p at free cols [blk*K, blk*K + K)
        w_sh = w[d0 : d0 + DH].reshape(NBLK, 128, K)
        wt = (
            w_sh.transpose(1, 0, 2).reshape(128, NBLK * K).astype(np.float32)
        )
        # diag(w_j) slabs for the PE blocks: partition p, col j*128 + m
        # holds w[blk*128+p, j] iff m == p else 0
        wdv = np.zeros((128, len(DIAG_BLKS), K, 128), dtype=np.float16)
        idx = np.arange(128)
        for i, blk in enumerate(DIAG_BLKS):
            wdv[idx, i, :, idx] = w_sh[blk].astype(np.float16)
        in_maps.append(
            {
                "xt": np.ascontiguousarray(xt),
                "wt": np.ascontiguousarray(wt),
                "wdt": np.ascontiguousarray(
                    wdv.reshape(128, len(DIAG_BLKS) * K * 128)
                ),
            }
        )
    return in_maps


def kernel(x, w):
    x = np.asarray(x, dtype=np.float32)
    w = np.asarray(w, dtype=np.float32)
    assert x.shape == (B, L, D) and w.shape == (D, K)

    if "nc" not in _cache:
        _cache["nc"] = _build_bass()
    nc = _cache["nc"]

    in_maps = _shard_inputs(x, w)
    res = None
    for attempt in range(3):
        try:
            res = run_bass_kernel_spmd(nc, in_maps, core_ids=list(range(N_CORES)))
            break
        except Exception:
            if attempt == 2:
                raise
    _cache["last_results"] = res

    out = np.empty((B, L, D), dtype=np.float32)
    for core in range(N_CORES):
        b, half = divmod(core, 2)
        d0 = half * DH
        out[b, :, d0 : d0 + DH] = res.results[core]["ot"].T.astype(np.float32)
    return out


# revision 16
# speedup vs baseline: 1.0557x; 1.0557x over previous
"""Depthwise causal Conv1d (k=4) + SiLU on 8 Trainium2 NeuronCores.

Problem: x [4, 4096, 2048] f32, w [2048, 4] f32,
out[b, t, d] = silu(sum_j w[d, j] * x[b, t - 3 + j, d])   (zero-padded left).

Sharding: 8 cores = 4 batches x 2 channel-halves. Depthwise conv is
independent per channel, so channel sharding needs no halo exchange.

Layout: each core receives its shard host-transposed to [channels, time]
(channels on SBUF partitions). The per-channel weight w[d, j] is a
per-partition scalar and the causal time shifts are free-dim AP offsets
into one loaded tile.

The kernel is HBM-bandwidth-bound: ~16.4 MB of fp16 I/O per core against
the ~27 GiB/s-per-SDMA-engine line rate (16 engines, ~44 us of streaming),
with compute only just fitting under the DMA window. Schedule:
 - All loads queue back to back at the head of the sync HWDGE ring
   (pure reads, no stalls); per-block stores queue on the SAME ring
   afterwards with plain data deps on their output tiles, so store
   descriptor-gen interleaves with the tail of the load stream and
   store HBM traffic overlaps compute instead of serializing after it.
 - diag(w_j) stationaries for the PE path are host-built and loaded
   as ONE packed 0.64 MB slab right after w (a single dispatch slot;
   building them on-chip costs critical DVE time, and GpSimd - though
   idle - must stay silent: any GpSimd streaming collides with DVE on
   the shared SBUF port and roughly halves DVE throughput).
 - Compute is spread so no engine exceeds the DMA window: blocks
   1,3,5,7 + half of 6 run on the TensorEngine as diag(w_j) matmuls
   accumulating the 4 taps in PSUM (j-inner per 512-col chunk,
   [128,1024] PSUM tiles 4 deep so the PE stays ahead of ACT's drain);
   blocks 0,2,4 + half of 6 run elementwise on DVE (shift-rebased
   products, pair-add tree), with the q1 product of the earliest
   halves on ACT (idle before SiLUs pile up). ACT does SiLU for
   everything: 1024-col chunks from PSUM, 2048-col from SBUF.

Precision: x and the output are host-cast fp16 (halves HBM traffic both
ways); products and adds stay fp16 (PE accumulates fp32 in PSUM); SiLU
computes fp32-internally on ACT. End-to-end relative error ~5e-4.
"""

import sys
import types

import numpy as np

import concourse.bass as bass
import concourse.bacc as bacc
import concourse.mybir as mybir
from concourse.tile import TileContext
from concourse.bass_utils import run_bass_kernel_spmd


def _ensure_ntff_hook():
    """bass_utils imports antenv.axon_hooks when BASS_TRACE is set; that
    module is absent on this image. Install a shim so tracing works when
    possible and degrades gracefully (instead of crashing) when not."""
    try:
        import antenv.axon_hooks  # noqa: F401

        return
    except ImportError:
        pass
    try:
        import antenv

        hook = None
        try:
            if "/root/.axon_site" not in sys.path:
                sys.path.insert(0, "/root/.axon_site")
            from trn_agent_boot.trn_boot import _ntff_profile_via_ctypes

            hook = _ntff_profile_via_ctypes("/opt/axon/libaxon_pjrt.so")
        except Exception:
            hook = None
        mod = types.ModuleType("antenv.axon_hooks")
        mod._hook = hook
        mod.get_axon_ntff_profile_hook = lambda: mod._hook
        mod.set_axon_ntff_profile_hook = lambda h: setattr(mod, "_hook", h)
        sys.modules["antenv.axon_hooks"] = mod
        antenv.axon_hooks = mod
    except Exception:
        pass


_ensure_ntff_hook()

B, L, D = 4, 4096, 2048
K = 4
PAD = K - 1
N_CORES = 8
DH = D // 2            # channels per core
NBLK = DH // 128       # 128-partition channel blocks per core
ROWW = 4128            # DRAM row stride (fp16 elems): 64B-aligned rows
HALF = L // 2
PQ = 1024              # PSUM tile width / PE silu chunk

MID_DT = mybir.dt.float16
PE_FULL = (1, 3, 5, 7)   # full blocks on the TensorEngine
PE_HALF_BLK = 6          # block 6: h0 on DVE, h1 on the PE
DIAG_BLKS = (1, 3, 5, 7, 6)

_cache = {}


def _build_bass():
    nc = bacc.Bacc()
    xt = nc.dram_tensor("xt", [DH, ROWW], MID_DT, kind="ExternalInput")
    wt = nc.dram_tensor("wt", [128, NBLK * K], mybir.dt.float32, kind="ExternalInput")
    # host-built diag(w_j) stationaries for the PE path, packed as one
    # [128, 5*K*128] fp16 slab (one DMA dispatch slot; building these
    # on-chip costs critical DVE time, and GpSimd - though idle - must
    # not run: any GpSimd activity collides with DVE on the shared SBUF
    # port and roughly halves DVE throughput while it streams).
    wdt = nc.dram_tensor(
        "wdt", [128, len(DIAG_BLKS) * K * 128], MID_DT, kind="ExternalInput"
    )
    ot = nc.dram_tensor("ot", [DH, L], MID_DT, kind="ExternalOutput")
    f32 = mybir.dt.float32
    silu = mybir.ActivationFunctionType.Silu

    with TileContext(nc) as tc:
        with tc.tile_pool(name="pool", bufs=2) as pool, \
             tc.tile_pool(name="psum", bufs=4, space="PSUM") as psum_pool:
            # w + diag slabs lead the sync ring so the PE can start as
            # soon as block 1's x lands.
            w = pool.tile([128, NBLK * K], f32, tag="w", bufs=1)
            nc.sync.dma_start(out=w[:], in_=wt[:, :])
            wd = pool.tile(
                [128, len(DIAG_BLKS) * K * 128], MID_DT, tag="wd", bufs=1
            )
            nc.sync.dma_start(out=wd[:], in_=wdt[:, :])
            # Warmup: a tiny Silu forces the silu activation-table set to
            # load during the initial DMA wait; it is the only table load
            # in the whole kernel.
            warm = pool.tile([128, 2], MID_DT, tag="warm", bufs=1)
            nc.vector.memset(warm[:], 0.0)
            nc.scalar.activation(warm[:], warm[:], silu)

            # All x loads up front, back to back on the sync ring.
            # Blocks 0 and 1 load in two pieces each, INTERLEAVED
            # (x0p1, x1p1, x0p2, x1p2) so both the DVE chain (block 0)
            # and the PE chain (block 1) get their first half ~1.5-2.5us
            # earlier than either whole tile would land. x7 loads before
            # x6 so the PE is fed in processing order.
            LOAD_ORDER = [0, 1, 2, 3, 4, 5, 7, 6]
            MIDCUT = HALF + PAD
            xts = {}
            for blk in LOAD_ORDER:
                xts[blk] = pool.tile(
                    [128, L + PAD + 1], MID_DT, tag="x", bufs=NBLK,
                    name=f"x{blk}",
                )
            for blk, t0, t1 in [
                (0, 0, MIDCUT), (1, 0, MIDCUT),
                (0, MIDCUT, L + PAD), (1, MIDCUT, L + PAD),
            ]:
                nc.sync.dma_start(
                    out=xts[blk][:, t0:t1],
                    in_=xt[blk * 128 : (blk + 1) * 128, t0:t1],
                )
            for blk in LOAD_ORDER[2:]:
                nc.sync.dma_start(
                    out=xts[blk][:, 0 : L + PAD],
                    in_=xt[blk * 128 : (blk + 1) * 128, 0 : L + PAD],
                )

            def lw_of(blk, j):
                c = (DIAG_BLKS.index(blk) * K + j) * 128
                return wd[:, c : c + 128]

            def pe_part(blk, x, o, t0, size):
                # TensorEngine path for [t0, t0+size): accumulate the 4
                # diag(w_j) matmuls per 512-col PSUM chunk (shift =
                # free-dim offset on the moving operand), SiLU from PSUM.
                # 1024-col PSUM quarters, 4 deep, so the PE stays 3 ahead
                # of ACT's PSUM drain.
                for q0 in range(t0, t0 + size, PQ):
                    ps = psum_pool.tile([128, PQ], f32, tag="ps", bufs=4)
                    for cc in range(PQ // 512):
                        for j in range(K):
                            nc.tensor.matmul(
                                ps[:, cc * 512 : (cc + 1) * 512],
                                lw_of(blk, j),
                                x[:, q0 + cc * 512 + j : q0 + cc * 512 + j + 512],
                                start=(j == 0),
                                stop=(j == K - 1),
                            )
                    nc.scalar.activation(
                        o[:, q0 : q0 + PQ], ps[:], silu
                    )

            def dve_half(blk, x, o, h0, n_act=0):
                # Elementwise path for [h0, h0+HALF): qe holds the
                # even-shift products [q0 | q2], qo the odd [q1 | q3]
                # (n_act of the odd ones on ACT to relieve DVE), pair-add
                # + final add on DVE, one 2048-col SiLU on ACT.
                # Shift-rebased: q_j[:, t] = w_j * x[:, h0 + t + j].
                wj = lambda j: w[:, blk * K + j : blk * K + j + 1]
                qe = pool.tile([128, 2, HALF], MID_DT, tag="qe", bufs=3)
                qo = pool.tile([128, 2, HALF], MID_DT, tag="qo", bufs=3)
                if n_act >= 1:
                    nc.scalar.mul(qo[:, 0, :], x[:, h0 + 1 : h0 + 1 + HALF], wj(1))
                else:
                    nc.vector.tensor_scalar_mul(
                        qo[:, 0, :], x[:, h0 + 1 : h0 + 1 + HALF], wj(1)
                    )
                nc.vector.tensor_scalar_mul(qe[:, 0, :], x[:, h0 : h0 + HALF], wj(0))
                if n_act >= 2:
                    nc.scalar.mul(qo[:, 1, :], x[:, h0 + 3 : h0 + 3 + HALF], wj(3))
                else:
                    nc.vector.tensor_scalar_mul(
                        qo[:, 1, :], x[:, h0 + 3 : h0 + 3 + HALF], wj(3)
                    )
                nc.vector.tensor_scalar_mul(qe[:, 1, :], x[:, h0 + 2 : h0 + 2 + HALF], wj(2))
                nc.vector.tensor_add(qe[:, :, :], qe[:, :, :], qo[:, :, :])
                nc.vector.tensor_add(qe[:, 0, :], qe[:, 0, :], qe[:, 1, :])
                nc.scalar.activation(o[:, h0 : h0 + HALF], qe[:, 0, :], silu)

            # Per-block compute. Emission order = per-engine queue order:
            # block 7 before 6 so the ACT SiLU queue tail matches
            # readiness (x6 is the last load). ACT takes the q1 product
            # of the earliest DVE halves (it is idle before SiLUs pile
            # up; DVE is the tighter budget).
            ACT_ODD = {(0, 0): 1, (0, 1): 1, (2, 0): 1}
            os_ = {}
            for blk in [0, 1, 2, 3, 4, 5, 7, 6]:
                x = xts[blk]
                o = pool.tile([128, L], MID_DT, tag="o", bufs=NBLK)
                os_[blk] = o
                if blk in PE_FULL:
                    pe_part(blk, x, o, 0, HALF)
                    pe_part(blk, x, o, HALF, HALF)
                elif blk == PE_HALF_BLK:
                    dve_half(blk, x, o, 0)
                    pe_part(blk, x, o, HALF, HALF)
                else:
                    for h in (0, 1):
                        dve_half(blk, x, o, h * HALF, ACT_ODD.get((blk, h), 0))

            # Stores: one half-block per dma_start, on the sync ring
            # behind all loads, each depending only on its output range
            # (last writer: that half's SiLUs). Half granularity lets
            # each half's bytes start draining as soon as it is done -
            # in particular the last block's h0 goes out ~5us before its
            # h1, shortening the final drain.
            for blk in [0, 1, 2, 3, 4, 5, 7, 6]:
                for h0 in (0, HALF):
                    nc.sync.dma_start(
                        out=ot[blk * 128 : (blk + 1) * 128, h0 : h0 + HALF],
                        in_=os_[blk][:, h0 : h0 + HALF],
                    )
    nc.compile()
    return nc


def _shard_inputs(x, w):
    in_maps = []
    for core in range(N_CORES):
        b, half = divmod(core, 2)
        d0 = half * DH
        xt = np.zeros((DH, ROWW), dtype=np.float16)
        xt[:, PAD : PAD + L] = x[b, :, d0 : d0 + DH].T.astype(np.float16)
        # w rows for this shard, rearranged so partition p holds the K
        # weights of channel blk*128 + p at free cols [blk*K, blk*K + K)
        w_sh = w[d0 : d0 + DH].reshape(NBLK, 128, K)
        wt = (
            w_sh.transpose(1, 0, 2).reshape(128, NBLK * K).astype(np.float32)
        )
        # diag(w_j) slabs for the PE blocks: partition p, col j*128 + m
        # holds w[blk*128+p, j] iff m == p else 0
        wdv = np.zeros((128, len(DIAG_BLKS), K, 128), dtype=np.float16)
        idx = np.arange(128)
        for i, blk in enumerate(DIAG_BLKS):
            wdv[idx, i, :, idx] = w_sh[blk].astype(np.float16)
        in_maps.append(
            {
                "xt": np.ascontiguousarray(xt),
                "wt": np.ascontiguousarray(wt),
                "wdt": np.ascontiguousarray(
                    wdv.reshape(128, len(DIAG_BLKS) * K * 128)
                ),
            }
        )
    return in_maps


def kernel(x, w):
    x = np.asarray(x, dtype=np.float32)
    w = np.asarray(w, dtype=np.float32)
    assert x.shape == (B, L, D) and w.shape == (D, K)

    if "nc" not in _cache:
        _cache["nc"] = _build_bass()
    nc = _cache["nc"]

    in_maps = _shard_inputs(x, w)
    res = None
    for attempt in range(3):
        try:
            res = run_bass_kernel_spmd(nc, in_maps, core_ids=list(range(N_CORES)))
            break
        except Exception:
            if attempt == 2:
                raise
    _cache["last_results"] = res

    out = np.empty((B, L, D), dtype=np.float32)
    for core in range(N_CORES):
        b, half = divmod(core, 2)
        d0 = half * DH
        out[b, :, d0 : d0 + DH] = res.results[core]["ot"].T.astype(np.float32)
    return out
